# revision 24
# baseline (speedup 1.0000x reference)
"""External Attention (nn_External_Attention) on 8 TRN2 NeuronCores.

kernel(x, Wk, Wv) -> x + Wv @ l1norm_M(softmax_N(Wk @ x))
  x  [16, 512, 4096] f32,  Wk [256, 512] f32,  Wv [512, 256] f32

Sharding: data-parallel over batch B=16 -> 2 batches per core across 8 cores.

Per core (C=512, M=256, N=4096), all PE matmuls fp8 DoubleRow (~340ns per
K=256 [128,512]-out unit measured on an 8-core power-realistic microbench):
  - x bf16 in / y bf16 out, stored as per-(batch, h) tiles [128, KC, 1024] so
    dependency tracking stays fine-grained.
  - x8 fp8e4 shadow via gpsimd software-DGE cast DMAs (SBUF->SBUF, one per
    (b, h) tile; no engine ALU cost), firing as soon as the 4 x loads land.
  - mm1: logits = (8*Wk)^T @ x8, K=512 as 2 accumulating DR matmuls per
    [128,512] half; exp on ACT -> E fp8e4 [128, KM, N] + f32 rowsums.
  - stats on DVE: rr = 1/rowsum; rrb_rep = fp8e4(4096*rr) replicated over the
    128 stationary columns; WVP = fp8e4(wvT*4096*rr).
  - cs+broadcast fused: rrb_rep^T @ E (DR) -> [128, XH] PSUM carries the
    column-sum in every partition; bc = exp(-ln(cs)) on ACT (ln and exp share
    one ACT table set -> the kernel never pays a table switch).
  - E' = E * bc -> fp8e5 [128, KM, XH]: t=0 plane on DVE, t=1 on Pool/gpsimd.
  - mm2: po = WVP^T @ E' (DR).  Evac y = po + x: most units add on DVE; every
    third unit accumulates identity@x on the PE and copies out on ACT.
  - PSUM: one shared pool (2x [128,1024]) serves mm1 logits AND cs tiles in
    rotation; po pool 2x [128,1024].  8 banks exactly.
  - Emission interleaves B(b0) with A(b1) per engine so the in-order queues
    can fill x-paced gaps of A(b1) with B(b0) work.
"""
from contextlib import ExitStack

import numpy as np
import ml_dtypes

import concourse.bacc as bacc
import concourse.bass as bass
import concourse.mybir as mybir
import concourse.tile as tile
from concourse.bass_utils import run_bass_kernel_spmd

F32 = mybir.dt.float32
BF16 = mybir.dt.bfloat16
FP8E4 = mybir.dt.float8e4
FP8E5 = mybir.dt.float8e5
AF = mybir.ActivationFunctionType
ALU = mybir.AluOpType
AX = mybir.AxisListType
DR = mybir.MatmulPerfMode.DoubleRow

B, C, M, N = 16, 512, 256, 4096
NCORES = 8
BPC = B // NCORES
NT = 512
KC = C // 128    # 4 x/channel tiles
KM = M // 128    # 2 memory tiles
NJ = N // NT     # 8 column tiles
XH = 1024
NH = N // XH     # 4 column-pair tiles
WK_SCALE = 8.0
S = 4096.0       # folded into wv upload and rrb; cancels in 1/cs


def _act_reciprocal(nc, out_ap, in_ap):
    """Direct InstActivation(Reciprocal) (bass helper bans it for precision;
    fine for the 3%-tolerant colsum normalizer).  Grouped emission keeps
    exp<->reciprocal ACT table switches to one pair per batch."""
    eng = nc.scalar
    inputs = [eng.lower_ap(in_ap),
              mybir.ImmediateValue(dtype=mybir.dt.float32, value=0.0),
              mybir.ImmediateValue(dtype=mybir.dt.float32, value=1.0),
              mybir.ImmediateValue(dtype=mybir.dt.float32, value=0.0)]
    return eng.add_instruction(
        mybir.InstActivation(
            name=nc.get_next_instruction_name(),
            func=AF.Reciprocal,
            ins=inputs,
            outs=[eng.lower_ap(out_ap)],
        )
    )


def _build(nc):
    x_d = nc.dram_tensor("x", [BPC, NH, 128, KC, XH], BF16, kind="ExternalInput").ap()
    x8_d = nc.dram_tensor("x8", [BPC, NH, 128, KC, XH], FP8E4, kind="ExternalInput").ap()
    wk_d = nc.dram_tensor("wk8", [128, KC, M], FP8E4, kind="ExternalInput").ap()
    wv_d = nc.dram_tensor("wvT", [128, KM, C], BF16, kind="ExternalInput").ap()
    id_d = nc.dram_tensor("ident", [128, 128], BF16, kind="ExternalInput").ap()
    y_d = nc.dram_tensor("y", [BPC, C, N], BF16, kind="ExternalOutput").ap()

    with tile.TileContext(nc) as tc, ExitStack() as ctx:
        wpool = ctx.enter_context(tc.tile_pool(name="w", bufs=1))
        xpool = ctx.enter_context(tc.tile_pool(name="xp", bufs=8))
        x8pool = ctx.enter_context(tc.tile_pool(name="x8p", bufs=8))
        epool = ctx.enter_context(tc.tile_pool(name="ep", bufs=2))
        eppool = ctx.enter_context(tc.tile_pool(name="epp", bufs=6))
        spool = ctx.enter_context(tc.tile_pool(name="sp", bufs=4))
        wvppool = ctx.enter_context(tc.tile_pool(name="wvp", bufs=2))
        ypool = ctx.enter_context(tc.tile_pool(name="yp", bufs=6))
        bcpool = ctx.enter_context(tc.tile_pool(name="bcp", bufs=2))
        lnpool = ctx.enter_context(tc.tile_pool(name="lnp", bufs=2))
        ps_l = ctx.enter_context(tc.tile_pool(name="ps_l", bufs=2, space="PSUM"))
        ps_o = ctx.enter_context(tc.tile_pool(name="ps_o", bufs=2, space="PSUM"))

        X = {b: [None] * NH for b in range(BPC)}
        X8 = {b: [None] * NH for b in range(BPC)}
        E, RSP, RRB, WVP, BC, EPT, YT = {}, {}, {}, {}, {}, {}, {}

        wk_sb = wpool.tile([128, KC, M], FP8E4, tag="wk", name="wk")
        nc.sync.dma_start(wk_sb[:], wk_d[:, :, :])

        def load_x8(b, h, nsplit=1):
            t = x8pool.tile([128, KC, XH], FP8E4, tag="x8", name=f"x8_{b}_{h}")
            ps = 128 // nsplit
            for i in range(nsplit):
                nc.sync.dma_start(
                    t[i * ps:(i + 1) * ps, :, :],
                    x8_d[b, h, i * ps:(i + 1) * ps, :, :])
            X8[b][h] = t

        def load_x(b, h):
            t = xpool.tile([128, KC, XH], BF16, tag="x", name=f"x{b}_{h}")
            nc.sync.dma_start(t[:], x_d[b, h, :, :, :])
            X[b][h] = t

        # x8 (mm1 operand) first, fine-grained at the head for low latency;
        # bf16 x later (only the evacuation needs it)
        load_x8(0, 0, nsplit=4)
        ebias = wpool.tile([128, 1], F32, tag="ebias", name="ebias")
        nc.vector.memset(ebias[:], -3.0)
        zbias = wpool.tile([128, 1], F32, tag="zbias", name="zbias")
        nc.vector.memset(zbias[:], 0.0)
        ones_sb = wpool.tile([128, 128], BF16, tag="ones", name="ones")
        nc.vector.memset(ones_sb[:], 1.0)
        for h in range(1, NH):
            load_x8(0, h, nsplit=2)
        for h in range(NH):
            load_x8(1, h, nsplit=2)
        wv_sb = wpool.tile([128, KM, C], BF16, tag="wv", name="wv")
        nc.sync.dma_start(wv_sb[:], wv_d[:, :, :])
        id_sb = wpool.tile([128, 128], BF16, tag="id", name="id")
        nc.sync.dma_start(id_sb[:], id_d[:, :])
        for h in range(NH):
            load_x(0, h)
        for h in range(NH):
            load_x(1, h)

        # ---- phase A units ----
        def emit_A_unit(b, h, mo):
            if E.get(b) is None:
                E[b] = epool.tile([128, KM, N], FP8E4, tag="e", name=f"e{b}")
                RSP[b] = [spool.tile([128, NH], F32, tag="rsp",
                                     name=f"rsp{b}_{m}") for m in range(KM)]
            pl = ps_l.tile([128, XH], F32, tag="psl", name=f"pl{b}_{h}_{mo}")
            for jj in range(2):
                for g in range(2):
                    nc.tensor.matmul(
                        pl[:, jj * NT:(jj + 1) * NT],
                        wk_sb[:, 2 * g:2 * g + 2, mo * 128:(mo + 1) * 128],
                        X8[b][h][:, 2 * g:2 * g + 2, jj * NT:(jj + 1) * NT],
                        start=(g == 0), stop=(g == 1), perf_mode=DR)
            nc.scalar.activation(
                E[b][:, mo, h * XH:(h + 1) * XH], pl[:],
                AF.Exp, bias=ebias[:], scale=1.0 / WK_SCALE,
                accum_out=RSP[b][mo][:, h:h + 1])

        def emit_stats(b):
            rrb = spool.tile([128, KM, 128], FP8E4, tag="rrb", name=f"rrb{b}")
            wvp = wvppool.tile([128, KM, C], FP8E4, tag="wvp", name=f"wvp{b}")
            for mo in range(KM):
                rs = spool.tile([128, 1], F32, tag="rs", name=f"rs{b}_{mo}")
                nc.vector.tensor_reduce(rs[:], RSP[b][mo][:], axis=AX.X, op=ALU.add)
                rr = spool.tile([128, 1], F32, tag="rr", name=f"rr{b}_{mo}")
                nc.vector.reciprocal(rr[:], rs[:])
                rrS = spool.tile([128, 1], F32, tag="rrS", name=f"rrS{b}_{mo}")
                nc.vector.tensor_scalar_mul(rrS[:], rr[:], S)
                nc.vector.tensor_scalar_mul(rrb[:, mo, :], ones_sb[:], rrS[:])
                nc.vector.tensor_scalar_mul(wvp[:, mo, :], wv_sb[:, mo, :], rr[:])
            RRB[b], WVP[b] = rrb, wvp

        # ---- phase B units ----
        def emit_cs_unit(b, jp):
            # fused colsum+broadcast; raw ACT Reciprocal -> bc bf16
            if jp == 0:
                BC[b] = bcpool.tile([128, N], BF16, tag="bcf", name=f"bcf{b}")
            cs = ps_l.tile([128, XH], F32, tag="psl", name=f"cs{b}_{jp}")
            for jj in range(2):
                j = 2 * jp + jj
                nc.tensor.matmul(cs[:, jj * NT:(jj + 1) * NT], RRB[b][:, :, :],
                                 E[b][:, :, j * NT:(j + 1) * NT],
                                 start=True, stop=True, perf_mode=DR)
            _act_reciprocal(nc, BC[b][:, jp * XH:(jp + 1) * XH], cs[:])

        def emit_ep(b, jp):
            # E' for the jp column-pair: one t-plane on DVE, one on Pool
            ep = eppool.tile([128, KM, XH], FP8E5, tag="epp", name=f"epp{b}_{jp}")
            sl = slice(jp * XH, (jp + 1) * XH)
            engs = (nc.vector, nc.gpsimd) if jp % 2 == 0 else (nc.gpsimd, nc.vector)
            for t in range(KM):
                engs[t].tensor_tensor(ep[:, t, :], E[b][:, t, sl],
                                      BC[b][:, sl], op=ALU.mult)
            EPT[(b, jp)] = ep

        def emit_mm2_unit(b, co, jp, pe_res):
            po = ps_o.tile([128, XH], F32, tag="po", name=f"po{b}_{co}_{jp}")
            for jj in range(2):
                nc.tensor.matmul(po[:, jj * NT:(jj + 1) * NT],
                                 WVP[b][:, :, co * 128:(co + 1) * 128],
                                 EPT[(b, jp)][:, :, jj * NT:(jj + 1) * NT],
                                 start=True, stop=not pe_res, perf_mode=DR)
                if pe_res:
                    nc.tensor.matmul(
                        po[:, jj * NT:(jj + 1) * NT], id_sb[:],
                        X[b][jp][:, co, jj * NT:(jj + 1) * NT],
                        start=False, stop=True)
            yt = ypool.tile([128, XH], BF16, tag="y", name=f"y{b}_{co}_{jp}")
            if pe_res:
                nc.scalar.copy(yt[:], po[:])
            else:
                nc.vector.tensor_tensor(yt[:], po[:], X[b][jp][:, co, :],
                                        op=ALU.add)
            nc.sync.dma_start(
                y_d[b, co * 128:(co + 1) * 128, jp * XH:(jp + 1) * XH], yt[:])

        def emit_mm2(b):
            # b1 runs in the drain tail: alternate DVE-add / ACT-copy evac
            # per co so two evacuation streams overlap (PE id is cheap there)
            for co in range(KC):
                for jp in range(NH):
                    pe_res = (jp == 3) if b == 1 else (co == 3 and jp >= 2)
                    emit_mm2_unit(b, co, jp, pe_res)

        # ---- emission schedule ----
        for h in range(NH):
            emit_A_unit(0, h, 0)
            emit_A_unit(0, h, 1)
        emit_stats(0)
        # B(b0) head before A(b1) so bc(b0) is not queued behind A(b1) exps
        for h in range(NH):
            emit_cs_unit(0, h)
            emit_ep(0, h)
        # interleave A(b1) units (x8-paced, gappy) with mm2(b0) units
        # (ready from ~18us) so the in-order PE queue stays dense
        mm2_units = [(c, p) for c in range(KC) for p in range(NH)]
        for h in range(NH):
            emit_A_unit(1, h, 0)
            emit_A_unit(1, h, 1)
            for co, jp in mm2_units[2 * h:2 * h + 2]:
                emit_mm2_unit(0, co, jp, pe_res=False)
        emit_stats(1)
        # fold B(b1)'s head into the mm2(b0) tail so mm2(b1) can start the
        # moment mm2(b0) drains
        for i, (co, jp) in enumerate(mm2_units[8:]):
            emit_mm2_unit(0, co, jp, pe_res=(co == 3 and jp >= 2))
            if i % 2 == 1 and i // 2 < NH:
                emit_cs_unit(1, i // 2)
                emit_ep(1, i // 2)
        emit_mm2(1)
    return nc


_CACHE = {}


def _get_program():
    if "nc" not in _CACHE:
        nc = bacc.Bacc("TRN2", target_bir_lowering=False, debug=False,
                       enable_asserts=True)
        _build(nc)
        nc.compile()
        _CACHE["nc"] = nc
    return _CACHE["nc"]


def _prep_inputs(x, Wk, Wv):
    xt = np.ascontiguousarray(
        np.asarray(x, dtype=np.float32).reshape(B, KC, 128, NH, XH)
        .transpose(0, 3, 2, 1, 4))
    xb = xt.astype(ml_dtypes.bfloat16)
    x8 = xt.astype(ml_dtypes.float8_e4m3)
    wk8 = np.ascontiguousarray(
        (np.asarray(Wk, dtype=np.float32).T * np.float32(WK_SCALE))
        .reshape(KC, 128, M).transpose(1, 0, 2)).astype(ml_dtypes.float8_e4m3)
    wvT = np.ascontiguousarray(
        (np.asarray(Wv, dtype=np.float32).T * np.float32(S))
        .reshape(KM, 128, C).transpose(1, 0, 2)).astype(ml_dtypes.bfloat16)
    ident = np.eye(128, dtype=np.float32).astype(ml_dtypes.bfloat16)
    return xb, x8, wk8, wvT, ident


def make_in_maps(x, Wk, Wv):
    xb, x8, wk8, wvT, ident = _prep_inputs(x, Wk, Wv)
    return [{"x": xb[i * BPC:(i + 1) * BPC], "x8": x8[i * BPC:(i + 1) * BPC],
             "wk8": wk8, "wvT": wvT, "ident": ident}
            for i in range(NCORES)]


def kernel(x, Wk, Wv):
    nc = _get_program()
    in_maps = make_in_maps(x, Wk, Wv)
    res = run_bass_kernel_spmd(nc, in_maps, list(range(NCORES)))
    y = np.concatenate([res.results[i]["y"] for i in range(NCORES)], axis=0)
    return np.ascontiguousarray(y.astype(np.float32))


# revision 25
# speedup vs baseline: 1.0540x; 1.0540x over previous
"""External Attention (nn_External_Attention) on 8 TRN2 NeuronCores.

kernel(x, Wk, Wv) -> x + Wv @ l1norm_M(softmax_N(Wk @ x))
  x  [16, 512, 4096] f32,  Wk [256, 512] f32,  Wv [512, 256] f32

Sharding: data-parallel over batch B=16 -> 2 batches per core across 8 cores.

Per core (C=512, M=256, N=4096), all PE matmuls fp8 DoubleRow (~340ns per
K=256 [128,512]-out unit measured on an 8-core power-realistic microbench):
  - x bf16 in / y bf16 out, stored as per-(batch, h) tiles [128, KC, 1024] so
    dependency tracking stays fine-grained.
  - x8 fp8e4 shadow via gpsimd software-DGE cast DMAs (SBUF->SBUF, one per
    (b, h) tile; no engine ALU cost), firing as soon as the 4 x loads land.
  - mm1: logits = (8*Wk)^T @ x8, K=512 as 2 accumulating DR matmuls per
    [128,512] half; exp on ACT -> E fp8e4 [128, KM, N] + f32 rowsums.
  - stats on DVE: rr = 1/rowsum; rrb_rep = fp8e4(4096*rr) replicated over the
    128 stationary columns; WVP = fp8e4(wvT*4096*rr).
  - cs+broadcast fused: rrb_rep^T @ E (DR) -> [128, XH] PSUM carries the
    column-sum in every partition; bc = exp(-ln(cs)) on ACT (ln and exp share
    one ACT table set -> the kernel never pays a table switch).
  - E' = E * bc -> fp8e5 [128, KM, XH]: t=0 plane on DVE, t=1 on Pool/gpsimd.
  - mm2: po = WVP^T @ E' (DR).  Evac y = po + x: most units add on DVE; every
    third unit accumulates identity@x on the PE and copies out on ACT.
  - PSUM: one shared pool (2x [128,1024]) serves mm1 logits AND cs tiles in
    rotation; po pool 2x [128,1024].  8 banks exactly.
  - Emission interleaves B(b0) with A(b1) per engine so the in-order queues
    can fill x-paced gaps of A(b1) with B(b0) work.
"""
from contextlib import ExitStack

import numpy as np
import ml_dtypes

import concourse.bacc as bacc
import concourse.bass as bass
import concourse.mybir as mybir
import concourse.tile as tile
from concourse.bass_utils import run_bass_kernel_spmd

F32 = mybir.dt.float32
BF16 = mybir.dt.bfloat16
FP8E4 = mybir.dt.float8e4
FP8E5 = mybir.dt.float8e5
AF = mybir.ActivationFunctionType
ALU = mybir.AluOpType
AX = mybir.AxisListType
DR = mybir.MatmulPerfMode.DoubleRow

B, C, M, N = 16, 512, 256, 4096
NCORES = 8
BPC = B // NCORES
NT = 512
KC = C // 128    # 4 x/channel tiles
KM = M // 128    # 2 memory tiles
NJ = N // NT     # 8 column tiles
XH = 1024
NH = N // XH     # 4 column-pair tiles
WK_SCALE = 8.0
S = 4096.0       # folded into wv upload and rrb; cancels in 1/cs


def _act_reciprocal(nc, out_ap, in_ap):
    """Direct InstActivation(Reciprocal) (bass helper bans it for precision;
    fine for the 3%-tolerant colsum normalizer).  Grouped emission keeps
    exp<->reciprocal ACT table switches to one pair per batch."""
    eng = nc.scalar
    inputs = [eng.lower_ap(in_ap),
              mybir.ImmediateValue(dtype=mybir.dt.float32, value=0.0),
              mybir.ImmediateValue(dtype=mybir.dt.float32, value=1.0),
              mybir.ImmediateValue(dtype=mybir.dt.float32, value=0.0)]
    return eng.add_instruction(
        mybir.InstActivation(
            name=nc.get_next_instruction_name(),
            func=AF.Reciprocal,
            ins=inputs,
            outs=[eng.lower_ap(out_ap)],
        )
    )


def _build(nc):
    x_d = nc.dram_tensor("x", [BPC, NH, 128, KC, XH], BF16, kind="ExternalInput").ap()
    x8_d = nc.dram_tensor("x8", [BPC, NH, 128, KC, XH], FP8E4, kind="ExternalInput").ap()
    wk_d = nc.dram_tensor("wk8", [128, KC, M], FP8E4, kind="ExternalInput").ap()
    wv_d = nc.dram_tensor("wvT", [128, KM, C], BF16, kind="ExternalInput").ap()
    id_d = nc.dram_tensor("ident", [128, 128], BF16, kind="ExternalInput").ap()
    y_d = nc.dram_tensor("y", [BPC, C, N], BF16, kind="ExternalOutput").ap()

    with tile.TileContext(nc) as tc, ExitStack() as ctx:
        wpool = ctx.enter_context(tc.tile_pool(name="w", bufs=1))
        xpool = ctx.enter_context(tc.tile_pool(name="xp", bufs=8))
        x8pool = ctx.enter_context(tc.tile_pool(name="x8p", bufs=8))
        epool = ctx.enter_context(tc.tile_pool(name="ep", bufs=2))
        eppool = ctx.enter_context(tc.tile_pool(name="epp", bufs=6))
        spool = ctx.enter_context(tc.tile_pool(name="sp", bufs=4))
        wvppool = ctx.enter_context(tc.tile_pool(name="wvp", bufs=2))
        ypool = ctx.enter_context(tc.tile_pool(name="yp", bufs=4))
        bcpool = ctx.enter_context(tc.tile_pool(name="bcp", bufs=2))
        lnpool = ctx.enter_context(tc.tile_pool(name="lnp", bufs=2))
        ps_l = ctx.enter_context(tc.tile_pool(name="ps_l", bufs=2, space="PSUM"))
        ps_o = ctx.enter_context(tc.tile_pool(name="ps_o", bufs=2, space="PSUM"))

        X = {b: [None] * NH for b in range(BPC)}
        X8 = {b: [None] * NH for b in range(BPC)}
        E, RSP, RRB, WVP, BC, EPT, YT = {}, {}, {}, {}, {}, {}, {}

        wk_sb = wpool.tile([128, KC, M], FP8E4, tag="wk", name="wk")
        nc.sync.dma_start(wk_sb[:], wk_d[:, :, :])

        def load_x8(b, h, nsplit=1):
            t = x8pool.tile([128, KC, XH], FP8E4, tag="x8", name=f"x8_{b}_{h}")
            ps = 128 // nsplit
            for i in range(nsplit):
                nc.sync.dma_start(
                    t[i * ps:(i + 1) * ps, :, :],
                    x8_d[b, h, i * ps:(i + 1) * ps, :, :])
            X8[b][h] = t

        def load_x(b, h):
            t = xpool.tile([128, KC, XH], BF16, tag="x", name=f"x{b}_{h}")
            nc.sync.dma_start(t[:], x_d[b, h, :, :, :])
            X[b][h] = t

        # x8 (mm1 operand) first, fine-grained at the head for low latency;
        # bf16 x later (only the evacuation needs it)
        load_x8(0, 0, nsplit=4)
        wv_sb = wpool.tile([128, KM, C], BF16, tag="wv", name="wv")
        nc.sync.dma_start(wv_sb[:], wv_d[:, :, :])
        id_sb = wpool.tile([128, 128], BF16, tag="id", name="id")
        nc.sync.dma_start(id_sb[:], id_d[:, :])
        ebias = wpool.tile([128, 1], F32, tag="ebias", name="ebias")
        nc.vector.memset(ebias[:], -3.0)
        zbias = wpool.tile([128, 1], F32, tag="zbias", name="zbias")
        nc.vector.memset(zbias[:], 0.0)
        ones_sb = wpool.tile([128, 128], BF16, tag="ones", name="ones")
        nc.vector.memset(ones_sb[:], 1.0)
        for h in range(1, NH):
            load_x8(0, h, nsplit=2)
        for h in range(NH):
            load_x8(1, h, nsplit=2)
        for h in range(NH):
            load_x(0, h)
        for h in range(NH):
            load_x(1, h)

        # ---- phase A units ----
        def emit_A_unit(b, h, mo):
            if E.get(b) is None:
                E[b] = epool.tile([128, KM, N], FP8E4, tag="e", name=f"e{b}")
                RSP[b] = [spool.tile([128, NH], F32, tag="rsp",
                                     name=f"rsp{b}_{m}") for m in range(KM)]
            pl = ps_l.tile([128, XH], F32, tag="psl", name=f"pl{b}_{h}_{mo}")
            for jj in range(2):
                for g in range(2):
                    nc.tensor.matmul(
                        pl[:, jj * NT:(jj + 1) * NT],
                        wk_sb[:, 2 * g:2 * g + 2, mo * 128:(mo + 1) * 128],
                        X8[b][h][:, 2 * g:2 * g + 2, jj * NT:(jj + 1) * NT],
                        start=(g == 0), stop=(g == 1), perf_mode=DR)
            nc.scalar.activation(
                E[b][:, mo, h * XH:(h + 1) * XH], pl[:],
                AF.Exp, bias=ebias[:], scale=1.0 / WK_SCALE,
                accum_out=RSP[b][mo][:, h:h + 1])

        def emit_stats(b):
            rrb = spool.tile([128, KM, 128], FP8E4, tag="rrb", name=f"rrb{b}")
            wvp = wvppool.tile([128, KM, C], FP8E4, tag="wvp", name=f"wvp{b}")
            for mo in range(KM):
                rs = spool.tile([128, 1], F32, tag="rs", name=f"rs{b}_{mo}")
                nc.vector.tensor_reduce(rs[:], RSP[b][mo][:], axis=AX.X, op=ALU.add)
                rr = spool.tile([128, 1], F32, tag="rr", name=f"rr{b}_{mo}")
                nc.vector.reciprocal(rr[:], rs[:])
                rrS = spool.tile([128, 1], F32, tag="rrS", name=f"rrS{b}_{mo}")
                nc.vector.tensor_scalar_mul(rrS[:], rr[:], S)
                nc.vector.tensor_scalar_mul(rrb[:, mo, :], ones_sb[:], rrS[:])
                nc.vector.tensor_scalar_mul(wvp[:, mo, :], wv_sb[:, mo, :], rr[:])
            RRB[b], WVP[b] = rrb, wvp

        # ---- phase B units ----
        def emit_cs_unit(b, jp):
            # fused colsum+broadcast; raw ACT Reciprocal -> bc bf16
            if jp == 0:
                BC[b] = bcpool.tile([128, N], BF16, tag="bcf", name=f"bcf{b}")
            cs = ps_l.tile([128, XH], F32, tag="psl", name=f"cs{b}_{jp}")
            for jj in range(2):
                j = 2 * jp + jj
                nc.tensor.matmul(cs[:, jj * NT:(jj + 1) * NT], RRB[b][:, :, :],
                                 E[b][:, :, j * NT:(j + 1) * NT],
                                 start=True, stop=True, perf_mode=DR)
            _act_reciprocal(nc, BC[b][:, jp * XH:(jp + 1) * XH], cs[:])

        def emit_ep(b, jp):
            # E' for the jp column-pair: one t-plane on DVE, one on Pool
            ep = eppool.tile([128, KM, XH], FP8E5, tag="epp", name=f"epp{b}_{jp}")
            sl = slice(jp * XH, (jp + 1) * XH)
            engs = (nc.vector, nc.gpsimd) if jp % 2 == 0 else (nc.gpsimd, nc.vector)
            for t in range(KM):
                engs[t].tensor_tensor(ep[:, t, :], E[b][:, t, sl],
                                      BC[b][:, sl], op=ALU.mult)
            EPT[(b, jp)] = ep

        def emit_mm2_unit(b, co, jp, pe_res):
            po = ps_o.tile([128, XH], F32, tag="po", name=f"po{b}_{co}_{jp}")
            for jj in range(2):
                nc.tensor.matmul(po[:, jj * NT:(jj + 1) * NT],
                                 WVP[b][:, :, co * 128:(co + 1) * 128],
                                 EPT[(b, jp)][:, :, jj * NT:(jj + 1) * NT],
                                 start=True, stop=not pe_res, perf_mode=DR)
                if pe_res:
                    nc.tensor.matmul(
                        po[:, jj * NT:(jj + 1) * NT], id_sb[:],
                        X[b][jp][:, co, jj * NT:(jj + 1) * NT],
                        start=False, stop=True)
            if jp % 2 == 0:
                YT[(b, co)] = ypool.tile([128, 2 * XH], BF16, tag="y",
                                         name=f"y{b}_{co}_{jp}")
            yt = YT[(b, co)][:, (jp % 2) * XH:(jp % 2 + 1) * XH]
            if pe_res:
                nc.scalar.copy(yt, po[:])
            else:
                nc.vector.tensor_tensor(yt, po[:], X[b][jp][:, co, :],
                                        op=ALU.add)
            if jp % 2 == 1:
                nc.sync.dma_start(
                    y_d[b, co * 128:(co + 1) * 128,
                        (jp - 1) * XH:(jp + 1) * XH], YT[(b, co)][:])

        def emit_mm2(b):
            # b1 runs in the drain tail: alternate DVE-add / ACT-copy evac
            # per co so two evacuation streams overlap (PE id is cheap there)
            for co in range(KC):
                for jp in range(NH):
                    pe_res = (jp == 3) if b == 1 else (co == 3 and jp >= 2)
                    emit_mm2_unit(b, co, jp, pe_res)

        # ---- emission schedule ----
        for h in range(NH):
            emit_A_unit(0, h, 0)
            emit_A_unit(0, h, 1)
        emit_stats(0)
        # B(b0) head before A(b1) so bc(b0) is not queued behind A(b1) exps
        for h in range(NH):
            emit_cs_unit(0, h)
            emit_ep(0, h)
        # interleave A(b1) units (x8-paced, gappy) with mm2(b0) units
        # (ready from ~18us) so the in-order PE queue stays dense
        mm2_units = [(c, p) for c in range(KC) for p in range(NH)]
        for h in range(NH):
            emit_A_unit(1, h, 0)
            emit_A_unit(1, h, 1)
            for co, jp in mm2_units[2 * h:2 * h + 2]:
                emit_mm2_unit(0, co, jp, pe_res=False)
        emit_stats(1)
        # fold B(b1)'s head into the mm2(b0) tail so mm2(b1) can start the
        # moment mm2(b0) drains
        for i, (co, jp) in enumerate(mm2_units[8:]):
            emit_mm2_unit(0, co, jp, pe_res=(co == 3 and jp >= 2))
            if i % 2 == 1 and i // 2 < NH:
                emit_cs_unit(1, i // 2)
                emit_ep(1, i // 2)
        emit_mm2(1)
    return nc


_CACHE = {}


def _get_program():
    if "nc" not in _CACHE:
        nc = bacc.Bacc("TRN2", target_bir_lowering=False, debug=False,
                       enable_asserts=True)
        _build(nc)
        nc.compile()
        _CACHE["nc"] = nc
    return _CACHE["nc"]


def _prep_inputs(x, Wk, Wv):
    xt = np.ascontiguousarray(
        np.asarray(x, dtype=np.float32).reshape(B, KC, 128, NH, XH)
        .transpose(0, 3, 2, 1, 4))
    xb = xt.astype(ml_dtypes.bfloat16)
    x8 = xt.astype(ml_dtypes.float8_e4m3)
    wk8 = np.ascontiguousarray(
        (np.asarray(Wk, dtype=np.float32).T * np.float32(WK_SCALE))
        .reshape(KC, 128, M).transpose(1, 0, 2)).astype(ml_dtypes.float8_e4m3)
    wvT = np.ascontiguousarray(
        (np.asarray(Wv, dtype=np.float32).T * np.float32(S))
        .reshape(KM, 128, C).transpose(1, 0, 2)).astype(ml_dtypes.bfloat16)
    ident = np.eye(128, dtype=np.float32).astype(ml_dtypes.bfloat16)
    return xb, x8, wk8, wvT, ident


def make_in_maps(x, Wk, Wv):
    xb, x8, wk8, wvT, ident = _prep_inputs(x, Wk, Wv)
    return [{"x": xb[i * BPC:(i + 1) * BPC], "x8": x8[i * BPC:(i + 1) * BPC],
             "wk8": wk8, "wvT": wvT, "ident": ident}
            for i in range(NCORES)]


def kernel(x, Wk, Wv):
    nc = _get_program()
    in_maps = make_in_maps(x, Wk, Wv)
    res = run_bass_kernel_spmd(nc, in_maps, list(range(NCORES)))
    y = np.concatenate([res.results[i]["y"] for i in range(NCORES)], axis=0)
    return np.ascontiguousarray(y.astype(np.float32))
